# revision 22
# baseline (speedup 1.0000x reference)
"""Trainium2 Bass kernel for nn_CausalAttention (GNN message passing).

Math (reference):
    pairs[e] = [img[:, src[e]] ; text[:, tgt[e]]]          # B == H == 128
    a[e]     = sigmoid(w2 . relu(W1 @ pairs[e] + b1) + b2) # per-edge gate
    att_img[b, i] = sum_{e: src[e]=i} a[e] * text[b, tgt[e]]
    att_txt[b, t] = sum_{e: tgt[e]=t} a[e] * img[b, src[e]]

Architecture: output-column sharding, fully on-chip. Core c owns
att_img[:, Wc] and att_txt[:, Wc], Wc = [128c, 128c+128).
All matmuls run fp16 (values) x fp8 (0/1 one-hot masks from host);
tolerance is 2e-2 so single fp16 precision is ample. For the img pipe
(txt pipe symmetric, roles swapped):
  - edges with src in Wc, bucketed by w = tgt >> 7 (8 fixed-capacity
    buckets of 5 blocks of 128 edge slots; unused slots are dummies).
  - tables in SBUF (fp16): txtT8[lo, w, b] = text[b, 128w+lo],
    V8[lo, w, h] = (W1_txt @ text).T likewise, UwinT[loc, h] for Wc.
  - per bucket (masks ohKT/ohLT fp8):
    h = relu(UwinT.T @ ohKT + V8[w].T @ ohLT + b1)     (PE matmuls)
    a = sigmoid(h.T @ w2 + b2)                         (per-block N=1 mm)
  - per bucket: ohKa[e, j, loc] = ohk * a (DVE, broadcast-AP multiply)
    M_w[lo, loc] += ohlo_j.T @ ohKa_j                  (PE, PSUM accum)
    att[:, loc] += txtT8[w].T @ M_w                    (PE)
Sides run sequentially so side-i mask gen (DVE) overlaps side-t's MLP
(PE). Host just concatenates the 8 column slices. Outputs f32.
"""

import sys

for _p in ("/opt/trn_rl_repo", "/root/.axon_site/_ro/trn_rl_repo"):
    if _p not in sys.path:
        sys.path.insert(0, _p)

import numpy as np
import ml_dtypes

import concourse.bass as bass
import concourse.tile as tile
from concourse import bacc, mybir

P = 128
DIM = 1024
E = 32768
NCORES = 8
NW = 8            # hi buckets
BPW = 5           # blocks per bucket (capacity 640 vs mean 512, +6 sigma)
NBLK = NW * BPW   # 40
EC = NBLK * P     # 5120 edge slots per pipeline
BW = BPW * P      # 640 edges per bucket

F32 = mybir.dt.float32
F16 = mybir.dt.float16
F8 = mybir.dt.float8e4
NP_F8 = ml_dtypes.float8_e4m3

MULT = mybir.AluOpType.mult
RELU = mybir.ActivationFunctionType.Relu
SIGM = mybir.ActivationFunctionType.Sigmoid


def _build_program():
    nc = bacc.Bacc(None, target_bir_lowering=False, debug=False)

    img16 = nc.dram_tensor("img16", [P, DIM], F16, kind="ExternalInput")
    txt16 = nc.dram_tensor("txt16", [P, DIM], F16, kind="ExternalInput")
    t8i_d = nc.dram_tensor("t8i_d", [P, DIM], F16, kind="ExternalInput")
    t8x_d = nc.dram_tensor("t8x_d", [P, DIM], F16, kind="ExternalInput")
    win_d = nc.dram_tensor("win_d", [P, 2 * P], F16, kind="ExternalInput")
    w1t_d = nc.dram_tensor("w1t_d", [P, 2 * P], F16, kind="ExternalInput")
    w2_d = nc.dram_tensor("w2_d", [P, 1], F16, kind="ExternalInput")
    cst_d = nc.dram_tensor("cst_d", [P, 2], F32, kind="ExternalInput")
    pin = {}
    for s in ("i", "t"):
        pin[s] = dict(
            ohkt=nc.dram_tensor(f"{s}_ohkt", [P, EC], F8, kind="ExternalInput"),
            ohlt=nc.dram_tensor(f"{s}_ohlt", [P, EC], F8, kind="ExternalInput"),
            ohlo=nc.dram_tensor(f"{s}_ohlo", [P, EC], F8, kind="ExternalInput"),
            ohk=nc.dram_tensor(f"{s}_ohk", [P, EC], F16, kind="ExternalInput"),
        )
    out_img = nc.dram_tensor("out_img", [P, P], F32, kind="ExternalOutput")
    out_txt = nc.dram_tensor("out_txt", [P, P], F32, kind="ExternalOutput")

    with tile.TileContext(nc) as tc:
        with (
            tc.tile_pool(name="const", bufs=1) as cp,
            tc.tile_pool(name="work", bufs=4) as wp,
            tc.tile_pool(name="psH", bufs=2, space="PSUM") as psH,
            tc.tile_pool(name="psM", bufs=1, space="PSUM") as psM,
            tc.tile_pool(name="psS", bufs=1, space="PSUM") as psS,
        ):
            # ---- constants / features ----
            img_s = cp.tile([P, DIM], F16)
            txt_s = cp.tile([P, DIM], F16)
            txtT8 = cp.tile([P, NW, P], F16)
            imgT8 = cp.tile([P, NW, P], F16)
            win_s = cp.tile([P, 2 * P], F16)
            w1t_s = cp.tile([P, 2 * P], F16)
            w2_s = cp.tile([P, 1], F16)
            cst_s = cp.tile([P, 2], F32)

            # prologue DMAs spread across the three DMA-capable queues so
            # the MLP-table builds can start as early as possible
            nc.scalar.dma_start(img_s[:], img16[:])
            nc.sync.dma_start(txt_s[:], txt16[:])
            nc.gpsimd.dma_start(w1t_s[:], w1t_d[:])
            nc.gpsimd.dma_start(win_s[:], win_d[:])
            nc.gpsimd.dma_start(cst_s[:], cst_d[:])
            nc.gpsimd.dma_start(w2_s[:], w2_d[:])
            b1_s = cst_s[:, 0:1]
            b2_s = cst_s[:, 1:2]
            imgw_s = win_s[:, :P]
            txtw_s = win_s[:, P:]
            w1i_s = w1t_s[:, :P]
            w1x_s = w1t_s[:, P:]

            # mask DMAs, chunked so bucket-0 compute starts early.
            NCH = 2
            CW = EC // NCH
            rep_s = {}
            for s in ("i", "t"):
                for k in ("ohkt", "ohlt", "ohlo", "ohk"):
                    t_ = cp.tile([P, EC], F16 if k == "ohk" else F8,
                                 tag=f"{s}{k}")
                    rep_s[(s, k)] = t_
            for ch in range(NCH):
                sl = slice(ch * CW, (ch + 1) * CW)
                nc.sync.dma_start(rep_s[("i", "ohkt")][:, sl], pin["i"]["ohkt"][:, sl])
                nc.sync.dma_start(rep_s[("i", "ohlt")][:, sl], pin["i"]["ohlt"][:, sl])
                nc.scalar.dma_start(rep_s[("t", "ohkt")][:, sl], pin["t"]["ohkt"][:, sl])
                nc.scalar.dma_start(rep_s[("t", "ohlt")][:, sl], pin["t"]["ohlt"][:, sl])
            nc.gpsimd.dma_start(rep_s[("i", "ohk")][:], pin["i"]["ohk"][:])
            nc.gpsimd.dma_start(rep_s[("i", "ohlo")][:], pin["i"]["ohlo"][:])
            nc.gpsimd.dma_start(rep_s[("t", "ohk")][:], pin["t"]["ohk"][:])
            nc.gpsimd.dma_start(rep_s[("t", "ohlo")][:], pin["t"]["ohlo"][:])
            nc.sync.dma_start(
                txtT8[:], t8x_d[:].rearrange("b (w lo) -> b w lo", lo=P))
            nc.sync.dma_start(
                imgT8[:], t8i_d[:].rearrange("b (w lo) -> b w lo", lo=P))

            # ---- prologue: build U8/V8 (all-node MLP tables) + window rows ----
            # U8[lo, w, h] = (W1_img @ img).T rows; UwinT likewise for Wc.
            U8 = cp.tile([P, NW, P], F16)
            V8 = cp.tile([P, NW, P], F16)
            UwinT = cp.tile([P, P], F16)
            VwinT = cp.tile([P, P], F16)

            def build(dst, lhsT, rhs):
                ps = psH.tile([P, BW], F32, tag="h_ps", name=f"b{id(dst)}")
                nc.tensor.matmul(ps[:, :P], lhsT, rhs, start=True, stop=True)
                nc.vector.tensor_copy(dst, ps[:, :P])

            build(UwinT[:], imgw_s, w1i_s)
            build(VwinT[:], txtw_s, w1x_s)
            for w in range(NW):
                build(V8[:, w, :], txt_s[:, w * P : (w + 1) * P], w1x_s)
            for w in range(NW):
                build(U8[:, w, :], img_s[:, w * P : (w + 1) * P], w1i_s)

            sides = {
                "i": dict(winT=UwinT, arbW=V8, arbT8=txtT8, out_d=out_img),
                "t": dict(winT=VwinT, arbW=U8, arbT8=imgT8, out_d=out_txt),
            }
            # single PSUM bank for all the small accumulators:
            # cols 0:128 acc-i, 128:256 acc-t, 256:296 a_ps-i, 296:336 a_ps-t
            psm = psS.tile([P, 336], F32)
            acc = {"i": psm[:, 0:P], "t": psm[:, P : 2 * P]}
            a_ps = {"i": psm[:, 2 * P : 2 * P + NBLK],
                    "t": psm[:, 2 * P + NBLK : 2 * P + 2 * NBLK]}
            h_ps = {}
            h_s = {}
            a_s = {}

            # ---- stage 1 (per side): per-edge MLP ----
            def phA(s, w):
                d = sides[s]
                ps = psH.tile([P, BW], F32, tag="h_ps", name=f"h_ps_{s}{w}")
                h_ps[(s, w)] = ps
                ohKT = rep_s[(s, "ohkt")][:, w * BW : (w + 1) * BW]
                ohLT = rep_s[(s, "ohlt")][:, w * BW : (w + 1) * BW]
                for mi, (st, oh_) in enumerate(((d["winT"][:], ohKT),
                                                (d["arbW"][:, w, :], ohLT))):
                    for o, n in ((0, 512), (512, P)):
                        nc.tensor.matmul(
                            ps[:, o : o + n], st, oh_[:, o : o + n],
                            start=(mi == 0), stop=(mi == 1),
                        )

            def relu(s, w):
                hs = wp.tile([P, BW], F16, tag="h_s", name=f"h_s_{s}{w}")
                h_s[(s, w)] = hs
                nc.scalar.activation(hs[:], h_ps[(s, w)][:], RELU, bias=b1_s)

            def aps(s, w):
                hs = h_s[(s, w)]
                for j in range(BPW):
                    b = w * BPW + j
                    nc.tensor.matmul(
                        a_ps[s][:, b : b + 1],
                        hs[:, j * P : (j + 1) * P], w2_s[:],
                        start=True, stop=True,
                    )

            def stage1(s):
                for w in range(NW):
                    phA(s, w)
                    relu(s, w)
                    if w >= 1:
                        aps(s, w - 1)
                aps(s, NW - 1)
                av = wp.tile([P, NBLK], F32, tag="a_s", name=f"a_s_{s}", bufs=2)
                a_s[s] = av
                nc.scalar.activation(av[:], a_ps[s][:], SIGM, bias=b2_s)

            # ---- per-bucket mask: ohKa = ohk * a (DVE broadcast multiply) ----
            ohKa = {}

            def mask(s, w):
                t_ = wp.tile([P, BPW, P], F16, tag="ohKa", bufs=4,
                             name=f"ohKa{s}{w}")
                ohKa[(s, w)] = t_
                for j in range(BPW):
                    b = w * BPW + j
                    ohk_blk = rep_s[(s, "ohk")][:, b * P : (b + 1) * P]
                    a_col = a_s[s][:, b : b + 1]
                    if j == 2:
                        nc.scalar.mul(t_[:, j, :], ohk_blk, a_col)
                    else:
                        nc.vector.tensor_scalar(
                            out=t_[:, j, :], in0=ohk_blk,
                            scalar1=a_col, scalar2=None, op0=MULT,
                        )

            # ---- stage 3 (per side): scatter (phase B) + tail ----
            def stage3(s):
                d = sides[s]
                ohlo_s = rep_s[(s, "ohlo")]
                m_ps0 = psM.tile([P, 4 * P], F32, tag="m0", name=f"m0{s}")
                m_ps1 = psM.tile([P, 4 * P], F32, tag="m1", name=f"m1{s}")
                m_ps = [m_ps0, m_ps1]
                for w in range(NW):
                    mslice = m_ps[w // 4][:, (w % 4) * P : (w % 4 + 1) * P]
                    ka = ohKa[(s, w)]
                    for j in range(BPW):
                        b = w * BPW + j
                        nc.tensor.matmul(
                            mslice, ohlo_s[:, b * P : (b + 1) * P],
                            ka[:, j, :],
                            start=(j == 0), stop=(j == BPW - 1),
                            skip_group_check=True,
                        )
                    m_s = wp.tile([P, P], F16, tag="m_s", bufs=4,
                                  name=f"m_s{s}{w}")
                    nc.scalar.copy(m_s[:], mslice)
                    nc.tensor.matmul(
                        acc[s][:], d["arbT8"][:, w, :], m_s[:],
                        start=(w == 0), stop=(w == NW - 1),
                        skip_group_check=True,
                    )
                out_sb = wp.tile([P, P], F32, tag="out_sb", bufs=2,
                                 name=f"out{s}")
                nc.vector.tensor_copy(out_sb[:], acc[s][:])
                nc.sync.dma_start(d["out_d"][:], out_sb[:])

            stage1("i")
            stage1("t")
            for w in range(NW):
                mask("i", w)
            for w in range(NW):
                mask("t", w)
            stage3("i")
            stage3("t")

    nc.compile()
    return nc


_PROGRAM = None


def _get_program():
    global _PROGRAM
    if _PROGRAM is None:
        _PROGRAM = _build_program()
    return _PROGRAM


def _pipe_arrays(key, arb, base):
    """key: bucketing key values (src for img pipe); arb: the other endpoint.
    Returns ohkt/ohlt/ohlo/ohk [P, EC] fp8 masks."""
    kloc = key - base                 # 0..127
    w = arb >> 7                      # bucket
    lo = arb & 127
    slots = np.full(EC, -1, np.int64)  # slot -> edge index or -1
    fill = np.zeros(NW, np.int64)
    order = np.argsort(w, kind="stable")
    for ei in order:
        wb = w[ei]
        assert fill[wb] < BW, f"bucket overflow: {fill[wb]}"
        slots[wb * BW + fill[wb]] = ei
        fill[wb] += 1
    klocs = np.full(EC, -1, np.int64)
    los = np.full(EC, -1, np.int64)
    used = slots >= 0
    klocs[used] = kloc[slots[used]]
    los[used] = lo[slots[used]]
    rng = np.arange(P)
    ohkt = np.ascontiguousarray((klocs[None, :] == rng[:, None]).astype(NP_F8))
    ohlt = np.ascontiguousarray((los[None, :] == rng[:, None]).astype(NP_F8))
    # ohlo[e % P, b*P + lo] = (los[e] == lo), block-diagonal [e, lo] tiles;
    # ohk likewise for klocs ([e, loc] layout)
    ohlo = np.zeros((P, EC), NP_F8)
    ohk = np.zeros((P, EC), np.float16)
    for b in range(NBLK):
        blk = los[b * P : (b + 1) * P]
        ohlo[:, b * P : (b + 1) * P] = (blk[:, None] == rng[None, :]).astype(NP_F8)
        blk = klocs[b * P : (b + 1) * P]
        ohk[:, b * P : (b + 1) * P] = blk[:, None] == rng[None, :]
    ohlo = np.ascontiguousarray(ohlo)
    ohk = np.ascontiguousarray(ohk)
    return ohkt, ohlt, ohlo, ohk


def _make_in_maps(img_features, text_features, src, tgt, W1, b1, w2, b2):
    img = np.asarray(img_features, dtype=np.float32)
    txt = np.asarray(text_features, dtype=np.float32)
    img16 = img.astype(np.float16)
    txt16 = txt.astype(np.float16)
    # t8x[lo, w*128 + b] = text[b, 128w + lo]  (txtT8 table, host-prearranged)
    t8x = np.ascontiguousarray(
        txt.T.reshape(NW, P, P).transpose(1, 0, 2).reshape(P, DIM)
    ).astype(np.float16)
    t8i = np.ascontiguousarray(
        img.T.reshape(NW, P, P).transpose(1, 0, 2).reshape(P, DIM)
    ).astype(np.float16)
    w1t = np.ascontiguousarray(
        np.concatenate([W1[:, :P].T, W1[:, P:].T], axis=1)
    ).astype(np.float16)
    b1c = np.asarray(b1, dtype=np.float32).reshape(P, 1)
    b2c = np.full((P, 1), np.float32(b2), dtype=np.float32)
    w2c = np.asarray(w2, dtype=np.float16).reshape(P, 1)
    src = np.asarray(src).astype(np.int64)
    tgt = np.asarray(tgt).astype(np.int64)

    in_maps = []
    for c in range(NCORES):
        base = c * P
        m = {
            "img16": img16, "txt16": txt16, "t8i_d": t8i, "t8x_d": t8x,
            "win_d": np.ascontiguousarray(np.concatenate(
                [img16[:, base : base + P], txt16[:, base : base + P]],
                axis=1)),
            "w1t_d": w1t, "w2_d": w2c,
            "cst_d": np.ascontiguousarray(np.concatenate([b1c, b2c], axis=1)),
        }
        for s, key, arb in (("i", src, tgt), ("t", tgt, src)):
            sel = (key >= base) & (key < base + P)
            ohkt, ohlt, ohlo, ohk = _pipe_arrays(key[sel], arb[sel], base)
            m[f"{s}_ohkt"] = ohkt
            m[f"{s}_ohlt"] = ohlt
            m[f"{s}_ohlo"] = ohlo
            m[f"{s}_ohk"] = ohk
        in_maps.append(m)
    return in_maps


def _run(inputs, trace=False):
    from concourse.bass_utils import run_bass_kernel_spmd

    nc = _get_program()
    in_maps = _make_in_maps(**inputs)
    res = run_bass_kernel_spmd(
        nc, in_maps, core_ids=list(range(NCORES)), trace=trace
    )
    att_img = np.concatenate([r["out_img"] for r in res.results], axis=1)
    att_txt = np.concatenate([r["out_txt"] for r in res.results], axis=1)
    return (np.ascontiguousarray(att_img), np.ascontiguousarray(att_txt)), res


def kernel(**inputs):
    out, _ = _run(inputs, trace=False)
    return out
